# revision 6
# baseline (speedup 1.0000x reference)
"""Causal self-attention (B=4, T=2048, C=1024, H=16) on 8 trn2 NeuronCores.

Sharding: core c handles batch b = c//2 and head-group g = c%2 (8 heads).
QKV/proj weights are split column/row-wise per head-group; each core returns
a partial projection output; the host sums the two head-group partials.

Per-core pipeline (all matmuls f32r = tf32-rate, attention probs fp16):
  A) QKV^T: qkv^T tiles from w-stationary matmuls (Q,K transposed [d,t]
     layout) + V in natural [t,d] layout from xT-stationary matmuls.
  B) per head: S^T[k,q] = K^T.T @ Q^T -> ACT exp (scale=1/8, causal via
     tile skip + triangular mask) -> A^T fp16 -> AV with ones-column
     appended to V giving y^T[d,q] and the softmax denominator row ->
     normalize via reciprocal + PE partition-broadcast.
  C) out = y^T.T @ w_proj accumulated over head pairs.
"""

import sys

sys.path.insert(0, "/opt/trn_rl_repo")

import numpy as np

import concourse.bass as bass
import concourse.mybir as mybir
import concourse.tile as tile
from concourse.bass_utils import run_bass_kernel_spmd

F32 = mybir.dt.float32
F32R = mybir.dt.float32r
F16 = mybir.dt.float16
EXP = mybir.ActivationFunctionType.Exp

T = 2048
C = 1024
NHL = 8  # local heads per core
DH = 64
NT = T // 128  # 16 t/k tiles
NCT = C // 128  # 8 contraction tiles
NQ = T // 512  # 4 q chunks
NK = T // 128  # 16 k tiles


def _split_multi_waits(nc):
    """walrus on this path encodes at most ONE sem-wait per instruction;
    hoist extra waits onto same-engine no-ops inserted just before."""
    for f in nc.m.functions:
        for bb in f.blocks:
            out = []
            changed = False
            for inst in bb.instructions:
                si = inst.sync_info
                ws = list(si.on_wait) if si is not None else []
                if len(ws) > 1:
                    changed = True
                    for j, w in enumerate(ws[:-1]):
                        nop = mybir.InstNoOp(name=f"{inst.name}-wsp{j}")
                        nop.engine = inst.engine
                        nop.sync_info = mybir.SyncInfo(on_wait=[w], on_update=[])
                        out.append(nop)
                    inst.sync_info = mybir.SyncInfo(
                        on_wait=[ws[-1]], on_update=list(si.on_update)
                    )
                out.append(inst)
            if changed:
                bb.instructions = out
    return nc


def _build():
    nc = bass.Bass(target_bir_lowering=False)
    xt_d = nc.declare_dram_parameter("xt", [C, T], F32R, isOutput=False)
    wqk_d = nc.declare_dram_parameter("wqk", [C, 1024], F32R, isOutput=False)
    wv_d = nc.declare_dram_parameter("wv", [C, 512], F32R, isOutput=False)
    wp_d = nc.declare_dram_parameter("wp", [512, C], F32R, isOutput=False)
    tri_d = nc.declare_dram_parameter("tri", [128, 128], F16, isOutput=False)
    ones_d = nc.declare_dram_parameter("ones64", [1, 64], F32R, isOutput=False)
    out_d = nc.declare_dram_parameter("out", [T, C], F32, isOutput=True)

    with tile.TileContext(nc) as tc:
        with (
            tc.tile_pool(name="qkt", bufs=1) as qkt_pool,
            tc.tile_pool(name="vsb", bufs=1) as v_pool,
            tc.tile_pool(name="ysb", bufs=1) as y_pool,
            tc.tile_pool(name="smalls", bufs=2) as small_pool,
            tc.tile_pool(name="consts", bufs=1) as const_pool,
        ):
            tri_sb = const_pool.tile([128, 128], F16, tag="tri", name="tri")
            nc.sync.dma_start(out=tri_sb[:, :], in_=tri_d.ap())
            ones_sb = const_pool.tile([1, 64], F32R, tag="ones", name="ones")
            nc.sync.dma_start(out=ones_sb[:, :], in_=ones_d.ap())

            # Q^T/K^T tiles [128(j), 2048(t)] f32r; j-tile p<4 -> Q heads
            # (2p, 2p+1); p>=4 -> K heads (2(p-4), 2(p-4)+1)
            qkt = [qkt_pool.tile([128, T], F32R, tag=f"qkt{j}", name=f"qkt{j}") for j in range(8)]
            # V tiles per k-tile: [128(t), 8*65] fp16, per-head 64 V cols + ones
            vt = [v_pool.tile([128, NHL * 65], F16, tag=f"v{k}", name=f"v{k}") for k in range(NT)]
            # y^T tiles [128(hd), 2048(t)] f32r, one per head pair
            yt = [y_pool.tile([128, T], F32R, tag=f"y{p}", name=f"y{p}") for p in range(4)]

            # ---------------- Phase A: QKV projections ----------------
            with tc.tile_pool(name="xt", bufs=1) as xt_pool:
                xt = [xt_pool.tile([128, T], F32R, tag=f"xt{ci}", name=f"xt{ci}") for ci in range(NCT)]
                for ci in range(NCT):
                    nc.sync.dma_start(
                        out=xt[ci][:, :], in_=xt_d.ap()[ci * 128 : (ci + 1) * 128, :]
                    )

                # Q^T / K^T: out[j(128), t] = w[c,j].T @ xT[c,t]
                with (
                    tc.tile_pool(name="pga", bufs=2, space="PSUM") as pga_pool,
                    tc.tile_pool(name="wqk", bufs=4) as wqk_pool,
                ):
                    for j in range(8):
                        pg = pga_pool.tile([128, T], F32, tag="pg", name=f"pg{j}")
                        for ci in range(NCT):
                            wt = wqk_pool.tile([128, 128], F32R, tag="w", name=f"w{j}_{ci}")
                            nc.sync.dma_start(
                                out=wt[:, :],
                                in_=wqk_d.ap()[
                                    ci * 128 : (ci + 1) * 128,
                                    j * 128 : (j + 1) * 128,
                                ],
                            )
                            for qc in range(NQ):
                                nc.tensor.matmul(
                                    pg[:, qc * 512 : (qc + 1) * 512],
                                    wt[:, :],
                                    xt[ci][:, qc * 512 : (qc + 1) * 512],
                                    start=(ci == 0),
                                    stop=(ci == NCT - 1),
                                )
                        nc.vector.tensor_copy(qkt[j][:, :], pg[:, :])

                # V natural layout: out[t(128), jv(512)] = xT[c,t].T @ wv[c,jv]
                with (
                    tc.tile_pool(name="pgv", bufs=3, space="PSUM") as pgv_pool,
                    tc.tile_pool(name="wv", bufs=1) as wv_pool,
                ):
                    wvt = [
                        wv_pool.tile([128, 512], F32R, tag=f"wv{ci}", name=f"wv{ci}")
                        for ci in range(NCT)
                    ]
                    for ci in range(NCT):
                        nc.sync.dma_start(
                            out=wvt[ci][:, :],
                            in_=wv_d.ap()[ci * 128 : (ci + 1) * 128, :],
                        )
                    for tt in range(NT):
                        pv = pgv_pool.tile([128, 512], F32, tag="pv", name=f"pv{tt}")
                        for ci in range(NCT):
                            nc.tensor.matmul(
                                pv[:, :],
                                xt[ci][:, tt * 128 : (tt + 1) * 128],
                                wvt[ci][:, :],
                                start=(ci == 0),
                                stop=(ci == NCT - 1),
                            )
                        v3 = vt[tt].rearrange("p (l c) -> p l c", c=65)
                        nc.vector.tensor_copy(
                            v3[:, :, 0:64],
                            pv[:, :].rearrange("p (l c) -> p l c", c=64),
                        )
                        nc.vector.memset(v3[:, :, 64:65], 1.0)

            # ---------------- Phase B: attention per head ----------------
            with tc.tile_pool(name="wp", bufs=1) as wp_pool:
                wpt = [wp_pool.tile([128, C], F32R, tag=f"wp{p}", name=f"wp{p}") for p in range(4)]
                for p in range(4):
                    nc.sync.dma_start(
                        out=wpt[p][:, :], in_=wp_d.ap()[p * 128 : (p + 1) * 128, :]
                    )

                with (
                    tc.tile_pool(name="apool", bufs=2) as a_pool,
                    tc.tile_pool(name="sg", bufs=1, space="PSUM") as sg_pool,
                    tc.tile_pool(name="yq", bufs=2, space="PSUM") as yq_pool,
                    tc.tile_pool(name="rbp", bufs=2, space="PSUM") as rbp_pool,
                ):
                    for h in range(NHL):
                        jq = h // 2
                        jk = 4 + h // 2
                        off = (h % 2) * 64
                        # -- pass 1: S^T tiles, exp, causal mask --
                        a_tiles = []
                        for k in range(NK):
                            width = T - 128 * k
                            sg = sg_pool.tile([128, T], F32, tag="sg", name=f"sg{h}_{k}")
                            for qc in range(k // 4, NQ):
                                q0 = max(qc * 512, k * 128)
                                q1 = (qc + 1) * 512
                                nc.tensor.matmul(
                                    sg[:, q0:q1],
                                    qkt[jk][off : off + 64, k * 128 : (k + 1) * 128],
                                    qkt[jq][off : off + 64, q0:q1],
                                    start=True,
                                    stop=True,
                                )
                            at = a_pool.tile([128, width], F16, tag=f"a{k}", name=f"a{h}_{k}")
                            nc.scalar.activation(
                                at[:, :], sg[:, 128 * k : T], EXP, scale=0.125
                            )
                            nc.vector.tensor_mul(
                                at[:, 0:128], at[:, 0:128], tri_sb[:, :]
                            )
                            a_tiles.append(at)
                        # -- pass 2: AV + denominator + normalize --
                        for qc in range(NQ):
                            yq = yq_pool.tile([65, 512], F32, tag="yq", name=f"yq{h}_{qc}")
                            klast = 4 * qc + 3
                            for k in range(klast + 1):
                                vsl = vt[k][:, h * 65 : (h + 1) * 65]
                                if k >= 4 * qc:  # diagonal tile
                                    n = 512 - (128 * k - 512 * qc)
                                    nc.tensor.matmul(
                                        yq[:, 512 - n : 512],
                                        vsl,
                                        a_tiles[k][:, 0:n],
                                        start=(k == 0),
                                        stop=(k == klast),
                                    )
                                else:
                                    c0 = qc * 512 - 128 * k
                                    nc.tensor.matmul(
                                        yq[:, :],
                                        vsl,
                                        a_tiles[k][:, c0 : c0 + 512],
                                        start=(k == 0),
                                        stop=(k == klast),
                                    )
                            r = small_pool.tile([1, 512], F32R, tag="recip", name=f"r{h}_{qc}")
                            with nc.allow_low_precision(reason="f32r is fp32-width"):
                                nc.vector.reciprocal(r[:, :], yq[64:65, :])
                            rbp = rbp_pool.tile([64, 512], F32, tag="rbp", name=f"rbp{h}_{qc}")
                            nc.tensor.matmul(
                                rbp[:, :], ones_sb[:, :], r[:, :], start=True, stop=True
                            )
                            rb = small_pool.tile([64, 512], F32, tag="rb", name=f"rb{h}_{qc}")
                            nc.vector.tensor_copy(rb[:, :], rbp[:, :])
                            nc.vector.tensor_mul(
                                yt[jq][off : off + 64, qc * 512 : (qc + 1) * 512],
                                yq[0:64, :],
                                rb[:, :],
                            )

                # ---------------- Phase C: output projection ----------------
                with (
                    tc.tile_pool(name="pj", bufs=2, space="PSUM") as pj_pool,
                    tc.tile_pool(name="ost", bufs=4) as ost_pool,
                ):
                    for tt in range(NT):
                        for jc in range(2):
                            pj = pj_pool.tile([128, 512], F32, tag="pj", name=f"pj{tt}_{jc}")
                            for p in range(4):
                                nc.tensor.matmul(
                                    pj[:, :],
                                    yt[p][:, tt * 128 : (tt + 1) * 128],
                                    wpt[p][:, jc * 512 : (jc + 1) * 512],
                                    start=(p == 0),
                                    stop=(p == 3),
                                )
                            ot = ost_pool.tile([128, 512], F32, tag="ost", name=f"ost{tt}_{jc}")
                            nc.scalar.copy(ot[:, :], pj[:, :])
                            nc.sync.dma_start(
                                out=out_d.ap()[
                                    tt * 128 : (tt + 1) * 128,
                                    jc * 512 : (jc + 1) * 512,
                                ],
                                in_=ot[:, :],
                            )

    _split_multi_waits(nc)
    return nc


_CACHED = {}


def _get_program():
    if "nc" not in _CACHED:
        _CACHED["nc"] = _build()
    return _CACHED["nc"]


def _shard_inputs(x, w_qkv, w_proj):
    x = np.ascontiguousarray(x, dtype=np.float32)
    w_qkv = np.ascontiguousarray(w_qkv, dtype=np.float32)
    w_proj = np.ascontiguousarray(w_proj, dtype=np.float32)
    tri = np.triu(np.ones((128, 128), dtype=np.float16))
    ones64 = np.ones((1, 64), dtype=np.float32)
    in_maps = []
    for core in range(8):
        b, g = core // 2, core % 2
        xt = np.ascontiguousarray(x[b].T)
        wqk = np.ascontiguousarray(
            np.concatenate(
                [
                    w_qkv[:, g * 512 : g * 512 + 512],
                    w_qkv[:, 1024 + g * 512 : 1024 + g * 512 + 512],
                ],
                axis=1,
            )
        )
        wv = np.ascontiguousarray(w_qkv[:, 2048 + g * 512 : 2048 + g * 512 + 512])
        wp = np.ascontiguousarray(w_proj[g * 512 : (g + 1) * 512, :])
        in_maps.append(
            {"xt": xt, "wqk": wqk, "wv": wv, "wp": wp, "tri": tri, "ones64": ones64}
        )
    return in_maps


def kernel(x, w_qkv, w_proj, _trace=False, _result_box=None):
    nc = _get_program()
    in_maps = _shard_inputs(x, w_qkv, w_proj)
    res = run_bass_kernel_spmd(nc, in_maps, list(range(8)), trace=_trace)
    if _result_box is not None:
        _result_box.append(res)
    B = x.shape[0]
    out = np.empty((B, T, C), dtype=np.float32)
    for b in range(B):
        out[b] = res.results[2 * b]["out"] + res.results[2 * b + 1]["out"]
    return out


# revision 19
# speedup vs baseline: 169.4213x; 169.4213x over previous
"""Causal self-attention (B=4, T=2048, C=1024, H=16) on 8 trn2 NeuronCores.

Sharding: core c handles batch b = c//2 and head-group g = c%2 (8 heads).
QKV/proj weights are split column/row-wise per head-group; each core returns
a partial projection output; the host sums the two head-group partials.

Per-core pipeline (all matmuls f32r = tf32-rate, attention probs fp16):
  A) QKV^T: qkv^T tiles from w-stationary matmuls (Q,K transposed [d,t]
     layout) + V in natural [t,d] layout from xT-stationary matmuls.
  B) per head: S^T[k,q] = K^T.T @ Q^T -> ACT exp (scale=1/8, causal via
     tile skip + triangular mask) -> A^T fp16 -> AV with ones-column
     appended to V giving y^T[d,q] and the softmax denominator row ->
     normalize via reciprocal + PE partition-broadcast.
  C) out = y^T.T @ w_proj accumulated over head pairs.
"""

import sys

sys.path.insert(0, "/opt/trn_rl_repo")

import numpy as np

import concourse.bass as bass
import concourse.mybir as mybir
import concourse.tile as tile
from concourse.bass_utils import run_bass_kernel_spmd

F32 = mybir.dt.float32
F32R = mybir.dt.float32r
F16 = mybir.dt.float16
EXP = mybir.ActivationFunctionType.Exp

T = 2048
C = 1024
NHL = 8  # local heads per core
DH = 64
NT = T // 128  # 16 t/k tiles
NCT = C // 128  # 8 contraction tiles
NQ = T // 512  # 4 q chunks
NK = T // 128  # 16 k tiles


def _split_multi_waits(nc):
    """walrus on this path encodes at most ONE sem-wait per instruction;
    hoist extra waits onto same-engine no-ops inserted just before."""
    for f in nc.m.functions:
        for bb in f.blocks:
            out = []
            changed = False
            for inst in bb.instructions:
                si = inst.sync_info
                ws = list(si.on_wait) if si is not None else []
                if len(ws) > 1:
                    changed = True
                    for j, w in enumerate(ws[:-1]):
                        nop = mybir.InstNoOp(name=f"{inst.name}-wsp{j}")
                        nop.engine = inst.engine
                        nop.sync_info = mybir.SyncInfo(on_wait=[w], on_update=[])
                        out.append(nop)
                    inst.sync_info = mybir.SyncInfo(
                        on_wait=[ws[-1]], on_update=list(si.on_update)
                    )
                out.append(inst)
            if changed:
                bb.instructions = out
    return nc


def _build(opts=None):
    opts = set(opts or ())
    nc = bass.Bass(target_bir_lowering=False)
    xt_d = nc.declare_dram_parameter("xt", [C, T], F32R, isOutput=False)
    wqk_d = nc.declare_dram_parameter("wqk", [C, 1024], F32R, isOutput=False)
    wv_d = nc.declare_dram_parameter("wv", [C, 512], F32R, isOutput=False)
    wp_d = nc.declare_dram_parameter("wp", [512, C], F32R, isOutput=False)
    tri_d = nc.declare_dram_parameter("tri", [128, 128], F16, isOutput=False)
    ones_d = nc.declare_dram_parameter("ones64", [1, 64], F32R, isOutput=False)
    out_d = nc.declare_dram_parameter("out", [T, C], F32, isOutput=True)

    with tile.TileContext(nc) as tc:
        with (
            tc.tile_pool(name="qkt", bufs=1) as qkt_pool,
            tc.tile_pool(name="vsb", bufs=1) as v_pool,
            tc.tile_pool(name="ysb", bufs=1) as y_pool,
            tc.tile_pool(name="smalls", bufs=2) as small_pool,
            tc.tile_pool(name="consts", bufs=1) as const_pool,
        ):
            tri_sb = const_pool.tile([128, 128], F16, tag="tri", name="tri")
            nc.sync.dma_start(out=tri_sb[:, :], in_=tri_d.ap())
            ones_sb = const_pool.tile([1, 64], F32R, tag="ones", name="ones")
            nc.sync.dma_start(out=ones_sb[:, :], in_=ones_d.ap())

            # Q^T/K^T tiles [128(j), 2048(t)] f32r; j-tile p<4 -> Q heads
            # (2p, 2p+1); p>=4 -> K heads (2(p-4), 2(p-4)+1)
            qkt = [qkt_pool.tile([128, T], F32R, tag=f"qkt{j}", name=f"qkt{j}") for j in range(8)]
            # V tiles per k-tile: [128(t), 8*65] fp16, per-head 64 V cols + ones
            vt = [v_pool.tile([128, NHL * 65], F16, tag=f"v{k}", name=f"v{k}") for k in range(NT)]
            # y^T tiles [128(hd), 2048(t)] f32r, one per head pair
            yt = (
                []
                if "onlyA" in opts
                else [
                    y_pool.tile([128, T], F32R, tag=f"y{p}", name=f"y{p}")
                    for p in range(4)
                ]
            )

            # ---------------- Phase A: QKV projections ----------------
            with tc.tile_pool(name="xt", bufs=1) as xt_pool:
                xt = [xt_pool.tile([128, T], F32R, tag=f"xt{ci}", name=f"xt{ci}") for ci in range(NCT)]
                for ci in range(NCT):
                    nc.sync.dma_start(
                        out=xt[ci][:, :], in_=xt_d.ap()[ci * 128 : (ci + 1) * 128, :]
                    )

                # Q^T / K^T: out[j(128), t] = w[c,j].T @ xT[c,t]
                with (
                    tc.tile_pool(name="pga", bufs=2, space="PSUM") as pga_pool,
                    tc.tile_pool(name="wqk", bufs=4) as wqk_pool,
                ):
                    for j in range(8):
                        pg = pga_pool.tile([128, T], F32, tag="pg", name=f"pg{j}")
                        for ci in range(1 if "qkvlite" in opts else NCT):
                            wt = wqk_pool.tile([128, 128], F32R, tag="w", name=f"w{j}_{ci}")
                            nc.sync.dma_start(
                                out=wt[:, :],
                                in_=wqk_d.ap()[
                                    ci * 128 : (ci + 1) * 128,
                                    j * 128 : (j + 1) * 128,
                                ],
                            )
                            for qc in range(NQ):
                                nc.tensor.matmul(
                                    pg[:, qc * 512 : (qc + 1) * 512],
                                    wt[:, :],
                                    xt[ci][:, qc * 512 : (qc + 1) * 512],
                                    start=(ci == 0),
                                    stop=(ci == NCT - 1) or "qkvlite" in opts,
                                )
                        nc.vector.tensor_copy(qkt[j][:, :], pg[:, :])

                # V natural layout: out[t(128), jv(512)] = xT[c,t].T @ wv[c,jv]
                with (
                    tc.tile_pool(name="pgv", bufs=3, space="PSUM") as pgv_pool,
                    tc.tile_pool(name="wv", bufs=1) as wv_pool,
                ):
                    wvt = [
                        wv_pool.tile([128, 512], F32R, tag=f"wv{ci}", name=f"wv{ci}")
                        for ci in range(NCT)
                    ]
                    for ci in range(NCT):
                        nc.sync.dma_start(
                            out=wvt[ci][:, :],
                            in_=wv_d.ap()[ci * 128 : (ci + 1) * 128, :],
                        )
                    for tt in range(NT):
                        pv = pgv_pool.tile([128, 512], F32, tag="pv", name=f"pv{tt}")
                        for ci in range(1 if "qkvlite" in opts else NCT):
                            nc.tensor.matmul(
                                pv[:, :],
                                xt[ci][:, tt * 128 : (tt + 1) * 128],
                                wvt[ci][:, :],
                                start=(ci == 0),
                                stop=(ci == NCT - 1) or "qkvlite" in opts,
                            )
                        v3 = vt[tt].rearrange("p (l c) -> p l c", c=65)
                        nc.vector.tensor_copy(
                            v3[:, :, 0:64],
                            pv[:, :].rearrange("p (l c) -> p l c", c=64),
                        )
                        nc.vector.memset(v3[:, :, 64:65], 1.0)

            # ---------------- Phase B: attention per head ----------------
            with tc.tile_pool(name="wp", bufs=1) as wp_pool:
                wpt = [wp_pool.tile([128, C], F32R, tag=f"wp{p}", name=f"wp{p}") for p in range(4)]
                for p in range(4):
                    nc.sync.dma_start(
                        out=wpt[p][:, :], in_=wp_d.ap()[p * 128 : (p + 1) * 128, :]
                    )

                with (
                    tc.tile_pool(name="apool", bufs=2) as a_pool,
                    tc.tile_pool(name="sg", bufs=1, space="PSUM") as sg_pool,
                    tc.tile_pool(name="yq", bufs=2, space="PSUM") as yq_pool,
                    tc.tile_pool(name="rbp", bufs=2, space="PSUM") as rbp_pool,
                ):
                    for h in range(0 if "onlyA" in opts else NHL):
                        jq = h // 2
                        jk = 4 + h // 2
                        off = (h % 2) * 64
                        # -- pass 1: S^T tiles, exp, causal mask --
                        a_tiles = []
                        for k in range(NK):
                            width = T - 128 * k
                            sg = sg_pool.tile([128, T], F32, tag="sg", name=f"sg{h}_{k}")
                            for qc in range(k // 4, (k // 4 + 1) if "stlite" in opts else NQ):
                                if "nost" in opts:
                                    break
                                q0 = max(qc * 512, k * 128)
                                q1 = (qc + 1) * 512
                                nc.tensor.matmul(
                                    sg[:, q0:q1],
                                    qkt[jk][off : off + 64, k * 128 : (k + 1) * 128],
                                    qkt[jq][off : off + 64, q0:q1],
                                    start=True,
                                    stop=True,
                                )
                            at = a_pool.tile([128, width], F16, tag=f"a{k}", name=f"a{h}_{k}")
                            if "noexp" in opts:
                                nc.vector.tensor_copy(at[:, 0:128], sg[:, 128 * k : 128 * k + 128])
                            else:
                                nc.scalar.activation(
                                    at[:, :], sg[:, 128 * k : T], EXP, scale=0.125
                                )
                            if "nomask" not in opts:
                                nc.vector.tensor_mul(
                                    at[:, 0:128], at[:, 0:128], tri_sb[:, :]
                                )
                            a_tiles.append(at)
                        # -- pass 2: AV + denominator + normalize --
                        for qc in range(NQ if "noav" not in opts else 0):
                            yq = yq_pool.tile([65, 512], F32, tag="yq", name=f"yq{h}_{qc}")
                            klast = 0 if "avlite" in opts else (4 * qc + 3)
                            for k in range(klast + 1):
                                vsl = vt[k][:, h * 65 : (h + 1) * 65]
                                if k >= 4 * qc:  # diagonal tile
                                    n = 512 - (128 * k - 512 * qc)
                                    nc.tensor.matmul(
                                        yq[:, 512 - n : 512],
                                        vsl,
                                        a_tiles[k][:, 0:n],
                                        start=(k == 0),
                                        stop=(k == klast),
                                    )
                                else:
                                    c0 = qc * 512 - 128 * k
                                    nc.tensor.matmul(
                                        yq[:, :],
                                        vsl,
                                        a_tiles[k][:, c0 : c0 + 512],
                                        start=(k == 0),
                                        stop=(k == klast),
                                    )
                            if "nonorm" in opts:
                                nc.vector.tensor_copy(
                                    yt[jq][off : off + 64, qc * 512 : (qc + 1) * 512],
                                    yq[0:64, :],
                                )
                            else:
                                r = small_pool.tile([1, 512], F32R, tag="recip", name=f"r{h}_{qc}")
                                with nc.allow_low_precision(reason="f32r is fp32-width"):
                                    nc.vector.reciprocal(r[:, :], yq[64:65, :])
                                rbp = rbp_pool.tile([64, 512], F32, tag="rbp", name=f"rbp{h}_{qc}")
                                nc.tensor.matmul(
                                    rbp[:, :], ones_sb[:, :], r[:, :], start=True, stop=True
                                )
                                rb = small_pool.tile([64, 512], F32, tag="rb", name=f"rb{h}_{qc}")
                                nc.vector.tensor_copy(rb[:, :], rbp[:, :])
                                nc.vector.tensor_mul(
                                    yt[jq][off : off + 64, qc * 512 : (qc + 1) * 512],
                                    yq[0:64, :],
                                    rb[:, :],
                                )

                # ---------------- Phase C: output projection ----------------
                with (
                    tc.tile_pool(name="pj", bufs=2, space="PSUM") as pj_pool,
                    tc.tile_pool(name="ost", bufs=4) as ost_pool,
                ):
                    for tt in range(0 if ("onlyA" in opts or "noC" in opts) else NT):
                        for jc in range(2):
                            pj = pj_pool.tile([128, 512], F32, tag="pj", name=f"pj{tt}_{jc}")
                            for p in range(4):
                                nc.tensor.matmul(
                                    pj[:, :],
                                    yt[p][:, tt * 128 : (tt + 1) * 128],
                                    wpt[p][:, jc * 512 : (jc + 1) * 512],
                                    start=(p == 0),
                                    stop=(p == 3),
                                )
                            ot = ost_pool.tile([128, 512], F32, tag="ost", name=f"ost{tt}_{jc}")
                            nc.scalar.copy(ot[:, :], pj[:, :])
                            nc.sync.dma_start(
                                out=out_d.ap()[
                                    tt * 128 : (tt + 1) * 128,
                                    jc * 512 : (jc + 1) * 512,
                                ],
                                in_=ot[:, :],
                            )

    _split_multi_waits(nc)
    return nc


_CACHED = {}


def _get_program():
    if "nc" not in _CACHED:
        _CACHED["nc"] = _build()
    return _CACHED["nc"]


def _shard_inputs(x, w_qkv, w_proj):
    x = np.ascontiguousarray(x, dtype=np.float32)
    w_qkv = np.ascontiguousarray(w_qkv, dtype=np.float32)
    w_proj = np.ascontiguousarray(w_proj, dtype=np.float32)
    tri = np.triu(np.ones((128, 128), dtype=np.float16))
    ones64 = np.ones((1, 64), dtype=np.float32)
    in_maps = []
    for core in range(8):
        b, g = core // 2, core % 2
        xt = np.ascontiguousarray(x[b].T)
        wqk = np.ascontiguousarray(
            np.concatenate(
                [
                    w_qkv[:, g * 512 : g * 512 + 512],
                    w_qkv[:, 1024 + g * 512 : 1024 + g * 512 + 512],
                ],
                axis=1,
            )
        )
        wv = np.ascontiguousarray(w_qkv[:, 2048 + g * 512 : 2048 + g * 512 + 512])
        wp = np.ascontiguousarray(w_proj[g * 512 : (g + 1) * 512, :])
        in_maps.append(
            {"xt": xt, "wqk": wqk, "wv": wv, "wp": wp, "tri": tri, "ones64": ones64}
        )
    return in_maps


def kernel(x, w_qkv, w_proj, _trace=False, _result_box=None):
    nc = _get_program()
    in_maps = _shard_inputs(x, w_qkv, w_proj)
    res = run_bass_kernel_spmd(nc, in_maps, list(range(8)), trace=_trace)
    if _result_box is not None:
        _result_box.append(res)
    B = x.shape[0]
    out = np.empty((B, T, C), dtype=np.float32)
    for b in range(B):
        out[b] = res.results[2 * b]["out"] + res.results[2 * b + 1]["out"]
    return out


# revision 25
# speedup vs baseline: 175.4940x; 1.0358x over previous
"""Causal self-attention (B=4, T=2048, C=1024, H=16) on 8 trn2 NeuronCores.

Sharding: core c handles batch b = c//2 and head-group g = c%2 (8 heads).
QKV/proj weights are split column/row-wise per head-group; each core returns
a partial projection output; the host sums the two head-group partials.

Per-core pipeline (all matmuls f32r = tf32-rate, attention probs fp16):
  A) QKV^T: qkv^T tiles from w-stationary matmuls (Q,K transposed [d,t]
     layout) + V in natural [t,d] layout from xT-stationary matmuls.
  B) per head: S^T[k,q] = K^T.T @ Q^T -> ACT exp (scale=1/8, causal via
     tile skip + triangular mask) -> A^T fp16 -> AV with ones-column
     appended to V giving y^T[d,q] and the softmax denominator row ->
     normalize via reciprocal + PE partition-broadcast.
  C) out = y^T.T @ w_proj accumulated over head pairs.
"""

import sys

sys.path.insert(0, "/opt/trn_rl_repo")

import numpy as np

import concourse.bass as bass
import concourse.mybir as mybir
import concourse.tile as tile
from concourse.bass_utils import run_bass_kernel_spmd

F32 = mybir.dt.float32
F32R = mybir.dt.float32r
F16 = mybir.dt.float16
EXP = mybir.ActivationFunctionType.Exp

T = 2048
C = 1024
NHL = 8  # local heads per core
DH = 64
NT = T // 128  # 16 t/k tiles
NCT = C // 128  # 8 contraction tiles
NQ = T // 512  # 4 q chunks
NK = T // 128  # 16 k tiles


def _split_multi_waits(nc):
    """walrus on this path encodes at most ONE sem-wait per instruction;
    hoist extra waits onto same-engine no-ops inserted just before."""
    for f in nc.m.functions:
        for bb in f.blocks:
            out = []
            changed = False
            for inst in bb.instructions:
                si = inst.sync_info
                ws = list(si.on_wait) if si is not None else []
                if len(ws) > 1:
                    changed = True
                    for j, w in enumerate(ws[:-1]):
                        nop = mybir.InstNoOp(name=f"{inst.name}-wsp{j}")
                        nop.engine = inst.engine
                        nop.sync_info = mybir.SyncInfo(on_wait=[w], on_update=[])
                        out.append(nop)
                    inst.sync_info = mybir.SyncInfo(
                        on_wait=[ws[-1]], on_update=list(si.on_update)
                    )
                out.append(inst)
            if changed:
                bb.instructions = out
    return nc


def _build(opts=None):
    opts = set(opts or ())
    nc = bass.Bass(target_bir_lowering=False)
    xt_d = nc.declare_dram_parameter("xt", [C, T], F32R, isOutput=False)
    wqk_d = nc.declare_dram_parameter("wqk", [C, 1024], F32R, isOutput=False)
    wv_d = nc.declare_dram_parameter("wv", [C, 512], F32R, isOutput=False)
    wp_d = nc.declare_dram_parameter("wp", [512, C], F32R, isOutput=False)
    tri_d = nc.declare_dram_parameter("tri", [128, 128], F16, isOutput=False)
    ones_d = nc.declare_dram_parameter("ones64", [1, 64], F32R, isOutput=False)
    out_d = nc.declare_dram_parameter("out", [T, C], F32, isOutput=True)

    with tile.TileContext(nc) as tc:
        with (
            tc.tile_pool(name="qkt", bufs=1) as qkt_pool,
            tc.tile_pool(name="vsb", bufs=1) as v_pool,
            tc.tile_pool(name="ysb", bufs=1) as y_pool,
            tc.tile_pool(name="smalls", bufs=2) as small_pool,
            tc.tile_pool(name="consts", bufs=1) as const_pool,
        ):
            tri_sb = const_pool.tile([128, 128], F16, tag="tri", name="tri")
            nc.sync.dma_start(out=tri_sb[:, :], in_=tri_d.ap())
            ones_sb = const_pool.tile([1, 64], F32R, tag="ones", name="ones")
            nc.sync.dma_start(out=ones_sb[:, :], in_=ones_d.ap())

            # Q^T/K^T tiles [128(j), 2048(t)] f32r; j-tile p<4 -> Q heads
            # (2p, 2p+1); p>=4 -> K heads (2(p-4), 2(p-4)+1)
            qkt = [qkt_pool.tile([128, T], F32R, tag=f"qkt{j}", name=f"qkt{j}") for j in range(8)]
            # V tiles per k-tile: [128(t), 8*65] fp16, per-head 64 V cols + ones
            vt = [v_pool.tile([128, NHL * 65], F16, tag=f"v{k}", name=f"v{k}") for k in range(NT)]
            # y^T tiles [128(hd), 2048(t)] f32r, one per head pair
            yt = (
                []
                if "onlyA" in opts
                else [
                    y_pool.tile([128, T], F32R, tag=f"y{p}", name=f"y{p}")
                    for p in range(4)
                ]
            )

            # ---------------- Phase A: QKV projections ----------------
            with tc.tile_pool(name="xt", bufs=1) as xt_pool:
                xt = [xt_pool.tile([128, T], F32R, tag=f"xt{ci}", name=f"xt{ci}") for ci in range(NCT)]
                for ci in range(NCT):
                    for qc in range(NQ):  # chunked: 4 DMAs/tile across queues
                        nc.sync.dma_start(
                            out=xt[ci][:, qc * 512 : (qc + 1) * 512],
                            in_=xt_d.ap()[
                                ci * 128 : (ci + 1) * 128, qc * 512 : (qc + 1) * 512
                            ],
                        )

                # Q^T / K^T: out[j(128), t] = w[c,j].T @ xT[c,t]
                with (
                    tc.tile_pool(name="pga", bufs=2, space="PSUM") as pga_pool,
                    tc.tile_pool(name="wqk", bufs=4) as wqk_pool,
                ):
                    for j in range(8):
                        pg = pga_pool.tile([128, T], F32, tag="pg", name=f"pg{j}")
                        for ci in range(1 if "qkvlite" in opts else NCT):
                            wt = wqk_pool.tile([128, 128], F32R, tag="w", name=f"w{j}_{ci}")
                            nc.sync.dma_start(
                                out=wt[:, :],
                                in_=wqk_d.ap()[
                                    ci * 128 : (ci + 1) * 128,
                                    j * 128 : (j + 1) * 128,
                                ],
                            )
                            for qc in range(NQ):
                                nc.tensor.matmul(
                                    pg[:, qc * 512 : (qc + 1) * 512],
                                    wt[:, :],
                                    xt[ci][:, qc * 512 : (qc + 1) * 512],
                                    start=(ci == 0),
                                    stop=(ci == NCT - 1) or "qkvlite" in opts,
                                )
                        nc.vector.tensor_copy(qkt[j][:, :], pg[:, :])

                # V natural layout: out[t(128), jv(512)] = xT[c,t].T @ wv[c,jv]
                with (
                    tc.tile_pool(name="pgv", bufs=3, space="PSUM") as pgv_pool,
                    tc.tile_pool(name="wv", bufs=1) as wv_pool,
                ):
                    wvt = [
                        wv_pool.tile([128, 512], F32R, tag=f"wv{ci}", name=f"wv{ci}")
                        for ci in range(NCT)
                    ]
                    for ci in range(NCT):
                        nc.sync.dma_start(
                            out=wvt[ci][:, :],
                            in_=wv_d.ap()[ci * 128 : (ci + 1) * 128, :],
                        )
                    for tt in range(NT):
                        pv = pgv_pool.tile([128, 512], F32, tag="pv", name=f"pv{tt}")
                        for ci in range(1 if "qkvlite" in opts else NCT):
                            nc.tensor.matmul(
                                pv[:, :],
                                xt[ci][:, tt * 128 : (tt + 1) * 128],
                                wvt[ci][:, :],
                                start=(ci == 0),
                                stop=(ci == NCT - 1) or "qkvlite" in opts,
                            )
                        v3 = vt[tt].rearrange("p (l c) -> p l c", c=65)
                        nc.vector.tensor_copy(
                            v3[:, :, 0:64],
                            pv[:, :].rearrange("p (l c) -> p l c", c=64),
                        )
                        nc.vector.memset(v3[:, :, 64:65], 1.0)

            # ---------------- Phase B: attention per head ----------------
            with tc.tile_pool(name="wp", bufs=1) as wp_pool:
                wpt = [wp_pool.tile([128, C], F32R, tag=f"wp{p}", name=f"wp{p}") for p in range(4)]
                for p in range(4):
                    nc.sync.dma_start(
                        out=wpt[p][:, :], in_=wp_d.ap()[p * 128 : (p + 1) * 128, :]
                    )

                with (
                    tc.tile_pool(name="apool", bufs=2) as a_pool,
                    tc.tile_pool(name="sg", bufs=1, space="PSUM") as sg_pool,
                    tc.tile_pool(name="yq", bufs=2, space="PSUM") as yq_pool,
                    tc.tile_pool(name="rbp", bufs=2, space="PSUM") as rbp_pool,
                ):
                    for h in range(0 if "onlyA" in opts else NHL):
                        jq = h // 2
                        jk = 4 + h // 2
                        off = (h % 2) * 64
                        # -- pass 1: S^T tiles, exp, causal mask --
                        a_tiles = []
                        for k in range(NK):
                            width = T - 128 * k
                            sg = sg_pool.tile([128, T], F32, tag="sg", name=f"sg{h}_{k}")
                            for qc in range(k // 4, (k // 4 + 1) if "stlite" in opts else NQ):
                                if "nost" in opts:
                                    break
                                q0 = max(qc * 512, k * 128)
                                q1 = (qc + 1) * 512
                                nc.tensor.matmul(
                                    sg[:, q0:q1],
                                    qkt[jk][off : off + 64, k * 128 : (k + 1) * 128],
                                    qkt[jq][off : off + 64, q0:q1],
                                    start=True,
                                    stop=True,
                                )
                            at = a_pool.tile([128, width], F16, tag=f"a{k}", name=f"a{h}_{k}")
                            if "noexp" in opts:
                                nc.vector.tensor_copy(at[:, 0:128], sg[:, 128 * k : 128 * k + 128])
                            else:
                                nc.scalar.activation(
                                    at[:, :], sg[:, 128 * k : T], EXP, scale=0.125
                                )
                            if "nomask" not in opts:
                                nc.vector.tensor_mul(
                                    at[:, 0:128], at[:, 0:128], tri_sb[:, :]
                                )
                            a_tiles.append(at)
                        # -- pass 2: AV + denominator + normalize --
                        for qc in range(NQ if "noav" not in opts else 0):
                            yq = yq_pool.tile([65, 512], F32, tag="yq", name=f"yq{h}_{qc}")
                            klast = 0 if "avlite" in opts else (4 * qc + 3)
                            for k in range(klast + 1):
                                vsl = vt[k][:, h * 65 : (h + 1) * 65]
                                if k >= 4 * qc:  # diagonal tile
                                    n = 512 - (128 * k - 512 * qc)
                                    nc.tensor.matmul(
                                        yq[:, 512 - n : 512],
                                        vsl,
                                        a_tiles[k][:, 0:n],
                                        start=(k == 0),
                                        stop=(k == klast),
                                    )
                                else:
                                    c0 = qc * 512 - 128 * k
                                    nc.tensor.matmul(
                                        yq[:, :],
                                        vsl,
                                        a_tiles[k][:, c0 : c0 + 512],
                                        start=(k == 0),
                                        stop=(k == klast),
                                    )
                            if "nonorm" in opts:
                                nc.vector.tensor_copy(
                                    yt[jq][off : off + 64, qc * 512 : (qc + 1) * 512],
                                    yq[0:64, :],
                                )
                            else:
                                r = small_pool.tile([1, 512], F32R, tag="recip", name=f"r{h}_{qc}")
                                with nc.allow_low_precision(reason="f32r is fp32-width"):
                                    nc.vector.reciprocal(r[:, :], yq[64:65, :])
                                rbp = rbp_pool.tile([64, 512], F32, tag="rbp", name=f"rbp{h}_{qc}")
                                nc.tensor.matmul(
                                    rbp[:, :], ones_sb[:, :], r[:, :], start=True, stop=True
                                )
                                rb = small_pool.tile([64, 512], F32, tag="rb", name=f"rb{h}_{qc}")
                                nc.vector.tensor_copy(rb[:, :], rbp[:, :])
                                nc.vector.tensor_mul(
                                    yt[jq][off : off + 64, qc * 512 : (qc + 1) * 512],
                                    yq[0:64, :],
                                    rb[:, :],
                                )

                # ---------------- Phase C: output projection ----------------
                with (
                    tc.tile_pool(name="pj", bufs=2, space="PSUM") as pj_pool,
                    tc.tile_pool(name="ost", bufs=4) as ost_pool,
                ):
                    for tt in range(0 if ("onlyA" in opts or "noC" in opts) else NT):
                        for jc in range(2):
                            pj = pj_pool.tile([128, 512], F32, tag="pj", name=f"pj{tt}_{jc}")
                            for p in range(4):
                                nc.tensor.matmul(
                                    pj[:, :],
                                    yt[p][:, tt * 128 : (tt + 1) * 128],
                                    wpt[p][:, jc * 512 : (jc + 1) * 512],
                                    start=(p == 0),
                                    stop=(p == 3),
                                )
                            ot = ost_pool.tile([128, 512], F32, tag="ost", name=f"ost{tt}_{jc}")
                            nc.scalar.copy(ot[:, :], pj[:, :])
                            nc.sync.dma_start(
                                out=out_d.ap()[
                                    tt * 128 : (tt + 1) * 128,
                                    jc * 512 : (jc + 1) * 512,
                                ],
                                in_=ot[:, :],
                            )

    _split_multi_waits(nc)
    return nc


_CACHED = {}


def _get_program():
    if "nc" not in _CACHED:
        _CACHED["nc"] = _build()
    return _CACHED["nc"]


def _shard_inputs(x, w_qkv, w_proj):
    x = np.ascontiguousarray(x, dtype=np.float32)
    w_qkv = np.ascontiguousarray(w_qkv, dtype=np.float32)
    w_proj = np.ascontiguousarray(w_proj, dtype=np.float32)
    tri = np.triu(np.ones((128, 128), dtype=np.float16))
    ones64 = np.ones((1, 64), dtype=np.float32)
    in_maps = []
    for core in range(8):
        b, g = core // 2, core % 2
        xt = np.ascontiguousarray(x[b].T)
        wqk = np.ascontiguousarray(
            np.concatenate(
                [
                    w_qkv[:, g * 512 : g * 512 + 512],
                    w_qkv[:, 1024 + g * 512 : 1024 + g * 512 + 512],
                ],
                axis=1,
            )
        )
        wv = np.ascontiguousarray(w_qkv[:, 2048 + g * 512 : 2048 + g * 512 + 512])
        wp = np.ascontiguousarray(w_proj[g * 512 : (g + 1) * 512, :])
        in_maps.append(
            {"xt": xt, "wqk": wqk, "wv": wv, "wp": wp, "tri": tri, "ones64": ones64}
        )
    return in_maps


def kernel(x, w_qkv, w_proj, _trace=False, _result_box=None):
    nc = _get_program()
    in_maps = _shard_inputs(x, w_qkv, w_proj)
    res = run_bass_kernel_spmd(nc, in_maps, list(range(8)), trace=_trace)
    if _result_box is not None:
        _result_box.append(res)
    B = x.shape[0]
    out = np.empty((B, T, C), dtype=np.float32)
    for b in range(B):
        out[b] = res.results[2 * b]["out"] + res.results[2 * b + 1]["out"]
    return out


# revision 27
# speedup vs baseline: 176.7851x; 1.0074x over previous
"""Causal self-attention (B=4, T=2048, C=1024, H=16) on 8 trn2 NeuronCores.

Sharding: core c handles batch b = c//2 and head-group g = c%2 (8 heads).
QKV/proj weights are split column/row-wise per head-group; each core returns
a partial projection output; the host sums the two head-group partials.

Per-core pipeline (all matmuls f32r = tf32-rate, attention probs fp16):
  A) QKV^T: qkv^T tiles from w-stationary matmuls (Q,K transposed [d,t]
     layout) + V in natural [t,d] layout from xT-stationary matmuls.
  B) per head: S^T[k,q] = K^T.T @ Q^T -> ACT exp (scale=1/8, causal via
     tile skip + triangular mask) -> A^T fp16 -> AV with ones-column
     appended to V giving y^T[d,q] and the softmax denominator row ->
     normalize via reciprocal + PE partition-broadcast.
  C) out = y^T.T @ w_proj accumulated over head pairs.
"""

import sys

sys.path.insert(0, "/opt/trn_rl_repo")

import numpy as np

import concourse.bass as bass
import concourse.mybir as mybir
import concourse.tile as tile
from concourse.bass_utils import run_bass_kernel_spmd

F32 = mybir.dt.float32
F32R = mybir.dt.float32r
F16 = mybir.dt.float16
EXP = mybir.ActivationFunctionType.Exp

T = 2048
C = 1024
NHL = 8  # local heads per core
DH = 64
NT = T // 128  # 16 t/k tiles
NCT = C // 128  # 8 contraction tiles
NQ = T // 512  # 4 q chunks
NK = T // 128  # 16 k tiles


def _split_multi_waits(nc):
    """walrus on this path encodes at most ONE sem-wait per instruction;
    hoist extra waits onto same-engine no-ops inserted just before."""
    for f in nc.m.functions:
        for bb in f.blocks:
            out = []
            changed = False
            for inst in bb.instructions:
                si = inst.sync_info
                ws = list(si.on_wait) if si is not None else []
                if len(ws) > 1:
                    changed = True
                    for j, w in enumerate(ws[:-1]):
                        nop = mybir.InstNoOp(name=f"{inst.name}-wsp{j}")
                        nop.engine = inst.engine
                        nop.sync_info = mybir.SyncInfo(on_wait=[w], on_update=[])
                        out.append(nop)
                    inst.sync_info = mybir.SyncInfo(
                        on_wait=[ws[-1]], on_update=list(si.on_update)
                    )
                out.append(inst)
            if changed:
                bb.instructions = out
    return nc


def _build(opts=None):
    opts = set(opts or ())
    nc = bass.Bass(target_bir_lowering=False)
    xt_d = nc.declare_dram_parameter("xt", [C, T], F32R, isOutput=False)
    wqk_d = nc.declare_dram_parameter("wqk", [C, 1024], F32R, isOutput=False)
    wv_d = nc.declare_dram_parameter("wv", [C, 512], F32R, isOutput=False)
    wp_d = nc.declare_dram_parameter("wp", [512, C], F32R, isOutput=False)
    tri_d = nc.declare_dram_parameter("tri", [128, 128], F16, isOutput=False)
    ones_d = nc.declare_dram_parameter("ones64", [1, 64], F32R, isOutput=False)
    out_d = nc.declare_dram_parameter("out", [T, C], F32, isOutput=True)

    with tile.TileContext(nc) as tc:
        with (
            tc.tile_pool(name="qkt", bufs=1) as qkt_pool,
            tc.tile_pool(name="vsb", bufs=1) as v_pool,
            tc.tile_pool(name="ysb", bufs=1) as y_pool,
            tc.tile_pool(name="smalls", bufs=2) as small_pool,
            tc.tile_pool(name="consts", bufs=1) as const_pool,
        ):
            tri_sb = const_pool.tile([128, 128], F16, tag="tri", name="tri")
            nc.sync.dma_start(out=tri_sb[:, :], in_=tri_d.ap())
            ones_sb = const_pool.tile([1, 64], F32R, tag="ones", name="ones")
            nc.sync.dma_start(out=ones_sb[:, :], in_=ones_d.ap())

            # Q^T/K^T tiles [128(j), 2048(t)] f32r; j-tile p<4 -> Q heads
            # (2p, 2p+1); p>=4 -> K heads (2(p-4), 2(p-4)+1)
            qkt = [qkt_pool.tile([128, T], F32R, tag=f"qkt{j}", name=f"qkt{j}") for j in range(8)]
            # V tiles per k-tile: [128(t), 8*65] fp16, per-head 64 V cols + ones
            vt = [v_pool.tile([128, NHL * 65], F16, tag=f"v{k}", name=f"v{k}") for k in range(NT)]
            # y^T tiles [128(hd), 2048(t)] f32r, one per head pair
            yt = (
                []
                if "onlyA" in opts
                else [
                    y_pool.tile([128, T], F32R, tag=f"y{p}", name=f"y{p}")
                    for p in range(4)
                ]
            )

            # ---------------- Phase A: QKV projections ----------------
            with tc.tile_pool(name="xt", bufs=1) as xt_pool:
                xt = [xt_pool.tile([128, T], F32R, tag=f"xt{ci}", name=f"xt{ci}") for ci in range(NCT)]
                for ci in range(NCT):
                    for hh in range(2):  # 2 chunks/tile: fewer HWDGE launches
                        nc.sync.dma_start(
                            out=xt[ci][:, hh * 1024 : (hh + 1) * 1024],
                            in_=xt_d.ap()[
                                ci * 128 : (ci + 1) * 128, hh * 1024 : (hh + 1) * 1024
                            ],
                        )

                # Q^T / K^T: out[j(128), t] = w[c,j].T @ xT[c,t]
                with (
                    tc.tile_pool(name="pga", bufs=2, space="PSUM") as pga_pool,
                    tc.tile_pool(name="wqk", bufs=4) as wqk_pool,
                ):
                    for j in range(8):
                        pg = pga_pool.tile([128, T], F32, tag="pg", name=f"pg{j}")
                        for ci in range(1 if "qkvlite" in opts else NCT):
                            wt = wqk_pool.tile([128, 128], F32R, tag="w", name=f"w{j}_{ci}")
                            nc.sync.dma_start(
                                out=wt[:, :],
                                in_=wqk_d.ap()[
                                    ci * 128 : (ci + 1) * 128,
                                    j * 128 : (j + 1) * 128,
                                ],
                            )
                            for qc in range(NQ):
                                nc.tensor.matmul(
                                    pg[:, qc * 512 : (qc + 1) * 512],
                                    wt[:, :],
                                    xt[ci][:, qc * 512 : (qc + 1) * 512],
                                    start=(ci == 0),
                                    stop=(ci == NCT - 1) or "qkvlite" in opts,
                                )
                        nc.vector.tensor_copy(qkt[j][:, :], pg[:, :])

                # V natural layout: out[t(128), jv(512)] = xT[c,t].T @ wv[c,jv]
                with (
                    tc.tile_pool(name="pgv", bufs=3, space="PSUM") as pgv_pool,
                    tc.tile_pool(name="wv", bufs=1) as wv_pool,
                ):
                    wvt = [
                        wv_pool.tile([128, 512], F32R, tag=f"wv{ci}", name=f"wv{ci}")
                        for ci in range(NCT)
                    ]
                    for ci in range(NCT):
                        nc.sync.dma_start(
                            out=wvt[ci][:, :],
                            in_=wv_d.ap()[ci * 128 : (ci + 1) * 128, :],
                        )
                    for tt in range(NT):
                        pv = pgv_pool.tile([128, 512], F32, tag="pv", name=f"pv{tt}")
                        for ci in range(1 if "qkvlite" in opts else NCT):
                            nc.tensor.matmul(
                                pv[:, :],
                                xt[ci][:, tt * 128 : (tt + 1) * 128],
                                wvt[ci][:, :],
                                start=(ci == 0),
                                stop=(ci == NCT - 1) or "qkvlite" in opts,
                            )
                        v3 = vt[tt].rearrange("p (l c) -> p l c", c=65)
                        nc.vector.tensor_copy(
                            v3[:, :, 0:64],
                            pv[:, :].rearrange("p (l c) -> p l c", c=64),
                        )
                        nc.vector.memset(v3[:, :, 64:65], 1.0)

            # ---------------- Phase B: attention per head ----------------
            with tc.tile_pool(name="wp", bufs=1) as wp_pool:
                wpt = [wp_pool.tile([128, C], F32R, tag=f"wp{p}", name=f"wp{p}") for p in range(4)]
                for p in range(4):
                    nc.sync.dma_start(
                        out=wpt[p][:, :], in_=wp_d.ap()[p * 128 : (p + 1) * 128, :]
                    )

                with (
                    tc.tile_pool(name="apool", bufs=2) as a_pool,
                    tc.tile_pool(name="sg", bufs=1, space="PSUM") as sg_pool,
                    tc.tile_pool(name="yq", bufs=2, space="PSUM") as yq_pool,
                    tc.tile_pool(name="rbp", bufs=2, space="PSUM") as rbp_pool,
                ):
                    for h in range(0 if "onlyA" in opts else NHL):
                        jq = h // 2
                        jk = 4 + h // 2
                        off = (h % 2) * 64
                        # -- pass 1: S^T tiles, exp, causal mask --
                        a_tiles = []
                        for k in range(NK):
                            width = T - 128 * k
                            sg = sg_pool.tile([128, T], F32, tag="sg", name=f"sg{h}_{k}")
                            for qc in range(k // 4, (k // 4 + 1) if "stlite" in opts else NQ):
                                if "nost" in opts:
                                    break
                                q0 = max(qc * 512, k * 128)
                                q1 = (qc + 1) * 512
                                nc.tensor.matmul(
                                    sg[:, q0:q1],
                                    qkt[jk][off : off + 64, k * 128 : (k + 1) * 128],
                                    qkt[jq][off : off + 64, q0:q1],
                                    start=True,
                                    stop=True,
                                )
                            at = a_pool.tile([128, width], F16, tag=f"a{k}", name=f"a{h}_{k}")
                            if "noexp" in opts:
                                nc.vector.tensor_copy(at[:, 0:128], sg[:, 128 * k : 128 * k + 128])
                            else:
                                nc.scalar.activation(
                                    at[:, :], sg[:, 128 * k : T], EXP, scale=0.125
                                )
                            if "nomask" not in opts:
                                nc.vector.tensor_mul(
                                    at[:, 0:128], at[:, 0:128], tri_sb[:, :]
                                )
                            a_tiles.append(at)
                        # -- pass 2: AV + denominator + normalize --
                        for qc in range(NQ if "noav" not in opts else 0):
                            yq = yq_pool.tile([65, 512], F32, tag="yq", name=f"yq{h}_{qc}")
                            klast = 0 if "avlite" in opts else (4 * qc + 3)
                            for k in range(klast + 1):
                                vsl = vt[k][:, h * 65 : (h + 1) * 65]
                                if k >= 4 * qc:  # diagonal tile
                                    n = 512 - (128 * k - 512 * qc)
                                    nc.tensor.matmul(
                                        yq[:, 512 - n : 512],
                                        vsl,
                                        a_tiles[k][:, 0:n],
                                        start=(k == 0),
                                        stop=(k == klast),
                                    )
                                else:
                                    c0 = qc * 512 - 128 * k
                                    nc.tensor.matmul(
                                        yq[:, :],
                                        vsl,
                                        a_tiles[k][:, c0 : c0 + 512],
                                        start=(k == 0),
                                        stop=(k == klast),
                                    )
                            if "nonorm" in opts:
                                nc.vector.tensor_copy(
                                    yt[jq][off : off + 64, qc * 512 : (qc + 1) * 512],
                                    yq[0:64, :],
                                )
                            else:
                                r = small_pool.tile([1, 512], F32R, tag="recip", name=f"r{h}_{qc}")
                                with nc.allow_low_precision(reason="f32r is fp32-width"):
                                    nc.vector.reciprocal(r[:, :], yq[64:65, :])
                                rbp = rbp_pool.tile([64, 512], F32, tag="rbp", name=f"rbp{h}_{qc}")
                                nc.tensor.matmul(
                                    rbp[:, :], ones_sb[:, :], r[:, :], start=True, stop=True
                                )
                                rb = small_pool.tile([64, 512], F32, tag="rb", name=f"rb{h}_{qc}")
                                nc.vector.tensor_copy(rb[:, :], rbp[:, :])
                                nc.vector.tensor_mul(
                                    yt[jq][off : off + 64, qc * 512 : (qc + 1) * 512],
                                    yq[0:64, :],
                                    rb[:, :],
                                )

                # ---------------- Phase C: output projection ----------------
                with (
                    tc.tile_pool(name="pj", bufs=2, space="PSUM") as pj_pool,
                    tc.tile_pool(name="ost", bufs=4) as ost_pool,
                ):
                    for tt in range(0 if ("onlyA" in opts or "noC" in opts) else NT):
                        for jc in range(2):
                            pj = pj_pool.tile([128, 512], F32, tag="pj", name=f"pj{tt}_{jc}")
                            for p in range(4):
                                nc.tensor.matmul(
                                    pj[:, :],
                                    yt[p][:, tt * 128 : (tt + 1) * 128],
                                    wpt[p][:, jc * 512 : (jc + 1) * 512],
                                    start=(p == 0),
                                    stop=(p == 3),
                                )
                            ot = ost_pool.tile([128, 512], F32, tag="ost", name=f"ost{tt}_{jc}")
                            nc.scalar.copy(ot[:, :], pj[:, :])
                            nc.sync.dma_start(
                                out=out_d.ap()[
                                    tt * 128 : (tt + 1) * 128,
                                    jc * 512 : (jc + 1) * 512,
                                ],
                                in_=ot[:, :],
                            )

    _split_multi_waits(nc)
    return nc


_CACHED = {}


def _get_program():
    if "nc" not in _CACHED:
        _CACHED["nc"] = _build()
    return _CACHED["nc"]


def _shard_inputs(x, w_qkv, w_proj):
    x = np.ascontiguousarray(x, dtype=np.float32)
    w_qkv = np.ascontiguousarray(w_qkv, dtype=np.float32)
    w_proj = np.ascontiguousarray(w_proj, dtype=np.float32)
    tri = np.triu(np.ones((128, 128), dtype=np.float16))
    ones64 = np.ones((1, 64), dtype=np.float32)
    in_maps = []
    for core in range(8):
        b, g = core // 2, core % 2
        xt = np.ascontiguousarray(x[b].T)
        wqk = np.ascontiguousarray(
            np.concatenate(
                [
                    w_qkv[:, g * 512 : g * 512 + 512],
                    w_qkv[:, 1024 + g * 512 : 1024 + g * 512 + 512],
                ],
                axis=1,
            )
        )
        wv = np.ascontiguousarray(w_qkv[:, 2048 + g * 512 : 2048 + g * 512 + 512])
        wp = np.ascontiguousarray(w_proj[g * 512 : (g + 1) * 512, :])
        in_maps.append(
            {"xt": xt, "wqk": wqk, "wv": wv, "wp": wp, "tri": tri, "ones64": ones64}
        )
    return in_maps


def kernel(x, w_qkv, w_proj, _trace=False, _result_box=None):
    nc = _get_program()
    in_maps = _shard_inputs(x, w_qkv, w_proj)
    res = run_bass_kernel_spmd(nc, in_maps, list(range(8)), trace=_trace)
    if _result_box is not None:
        _result_box.append(res)
    B = x.shape[0]
    out = np.empty((B, T, C), dtype=np.float32)
    for b in range(B):
        out[b] = res.results[2 * b]["out"] + res.results[2 * b + 1]["out"]
    return out


# revision 30
# speedup vs baseline: 177.6992x; 1.0052x over previous
"""Causal self-attention (B=4, T=2048, C=1024, H=16) on 8 trn2 NeuronCores.

Sharding: core c handles batch b = c//2 and head-group g = c%2 (8 heads).
QKV/proj weights are split column/row-wise per head-group; each core returns
a partial projection output; the host sums the two head-group partials.

Per-core pipeline (all matmuls f32r = tf32-rate, attention probs fp16):
  A) QKV^T: qkv^T tiles from w-stationary matmuls (Q,K transposed [d,t]
     layout) + V in natural [t,d] layout from xT-stationary matmuls.
  B) per head: S^T[k,q] = K^T.T @ Q^T -> ACT exp (scale=1/8, causal via
     tile skip + triangular mask) -> A^T fp16 -> AV with ones-column
     appended to V giving y^T[d,q] and the softmax denominator row ->
     normalize via reciprocal + PE partition-broadcast.
  C) out = y^T.T @ w_proj accumulated over head pairs.
"""

import sys

sys.path.insert(0, "/opt/trn_rl_repo")

import numpy as np

import concourse.bass as bass
import concourse.mybir as mybir
import concourse.tile as tile
from concourse.bass_utils import run_bass_kernel_spmd

F32 = mybir.dt.float32
F32R = mybir.dt.float32r
F16 = mybir.dt.float16
EXP = mybir.ActivationFunctionType.Exp

T = 2048
C = 1024
NHL = 8  # local heads per core
DH = 64
NT = T // 128  # 16 t/k tiles
NCT = C // 128  # 8 contraction tiles
NQ = T // 512  # 4 q chunks
NK = T // 128  # 16 k tiles


def _split_multi_waits(nc):
    """walrus on this path encodes at most ONE sem-wait per instruction;
    hoist extra waits onto same-engine no-ops inserted just before."""
    for f in nc.m.functions:
        for bb in f.blocks:
            out = []
            changed = False
            for inst in bb.instructions:
                si = inst.sync_info
                ws = list(si.on_wait) if si is not None else []
                if len(ws) > 1:
                    changed = True
                    for j, w in enumerate(ws[:-1]):
                        nop = mybir.InstNoOp(name=f"{inst.name}-wsp{j}")
                        nop.engine = inst.engine
                        nop.sync_info = mybir.SyncInfo(on_wait=[w], on_update=[])
                        out.append(nop)
                    inst.sync_info = mybir.SyncInfo(
                        on_wait=[ws[-1]], on_update=list(si.on_update)
                    )
                out.append(inst)
            if changed:
                bb.instructions = out
    return nc


def _build(opts=None):
    opts = set(opts or ())
    nc = bass.Bass(target_bir_lowering=False)
    xt_d = nc.declare_dram_parameter("xt", [C, T], F32R, isOutput=False)
    wqk_d = nc.declare_dram_parameter("wqk", [C, 1024], F32R, isOutput=False)
    wv_d = nc.declare_dram_parameter("wv", [C, 512], F32R, isOutput=False)
    wp_d = nc.declare_dram_parameter("wp", [512, C], F32R, isOutput=False)
    tri_d = nc.declare_dram_parameter("tri", [128, 128], F16, isOutput=False)
    ones_d = nc.declare_dram_parameter("ones64", [1, 64], F32R, isOutput=False)
    out_d = nc.declare_dram_parameter("out", [T, C], F32, isOutput=True)

    with tile.TileContext(nc) as tc:
        with (
            tc.tile_pool(name="qkt", bufs=1) as qkt_pool,
            tc.tile_pool(name="vsb", bufs=1) as v_pool,
            tc.tile_pool(name="ysb", bufs=1) as y_pool,
            tc.tile_pool(name="smalls", bufs=2) as small_pool,
            tc.tile_pool(name="consts", bufs=1) as const_pool,
        ):
            tri_sb = const_pool.tile([128, 128], F16, tag="tri", name="tri")
            nc.sync.dma_start(out=tri_sb[:, :], in_=tri_d.ap())
            ones_sb = const_pool.tile([1, 64], F32R, tag="ones", name="ones")
            nc.sync.dma_start(out=ones_sb[:, :], in_=ones_d.ap())

            # Q^T/K^T tiles [128(j), 2048(t)] f32r; j-tile p<4 -> Q heads
            # (2p, 2p+1); p>=4 -> K heads (2(p-4), 2(p-4)+1)
            qkt = [qkt_pool.tile([128, T], F32R, tag=f"qkt{j}", name=f"qkt{j}") for j in range(8)]
            # V tiles per k-tile: [128(t), 8*65] fp16, per-head 64 V cols + ones
            vt = [v_pool.tile([128, NHL * 65], F16, tag=f"v{k}", name=f"v{k}") for k in range(NT)]
            # y^T tiles [128(hd), 2048(t)] f32r, one per head pair
            yt = (
                []
                if "onlyA" in opts
                else [
                    y_pool.tile([128, T], F32R, tag=f"y{p}", name=f"y{p}")
                    for p in range(4)
                ]
            )

            # ---------------- Phase A: QKV projections ----------------
            with tc.tile_pool(name="xt", bufs=1) as xt_pool:
                xt = [xt_pool.tile([128, T], F32R, tag=f"xt{ci}", name=f"xt{ci}") for ci in range(NCT)]
                for ci in range(NCT):
                    for hh in range(2):  # 2 chunks/tile: fewer HWDGE launches
                        nc.sync.dma_start(
                            out=xt[ci][:, hh * 1024 : (hh + 1) * 1024],
                            in_=xt_d.ap()[
                                ci * 128 : (ci + 1) * 128, hh * 1024 : (hh + 1) * 1024
                            ],
                        )

                # Q^T / K^T: out[j(128), t] = w[c,j].T @ xT[c,t]
                with (
                    tc.tile_pool(name="pga", bufs=2, space="PSUM") as pga_pool,
                    tc.tile_pool(name="wqk", bufs=4) as wqk_pool,
                ):
                    for j in range(8):
                        pg = pga_pool.tile([128, T], F32, tag="pg", name=f"pg{j}")
                        for ci in range(1 if "qkvlite" in opts else NCT):
                            wt = wqk_pool.tile([128, 128], F32R, tag="w", name=f"w{j}_{ci}")
                            nc.sync.dma_start(
                                out=wt[:, :],
                                in_=wqk_d.ap()[
                                    ci * 128 : (ci + 1) * 128,
                                    j * 128 : (j + 1) * 128,
                                ],
                            )
                            for qc in range(NQ):
                                nc.tensor.matmul(
                                    pg[:, qc * 512 : (qc + 1) * 512],
                                    wt[:, :],
                                    xt[ci][:, qc * 512 : (qc + 1) * 512],
                                    start=(ci == 0),
                                    stop=(ci == NCT - 1) or "qkvlite" in opts,
                                )
                        nc.vector.tensor_copy(qkt[j][:, :], pg[:, :])

                # V natural layout: out[t(128), jv(512)] = xT[c,t].T @ wv[c,jv]
                with (
                    tc.tile_pool(name="pgv", bufs=4, space="PSUM") as pgv_pool,
                    tc.tile_pool(name="wv", bufs=1) as wv_pool,
                ):
                    wvt = [
                        wv_pool.tile([128, 512], F32R, tag=f"wv{ci}", name=f"wv{ci}")
                        for ci in range(NCT)
                    ]
                    for ci in range(NCT):
                        nc.sync.dma_start(
                            out=wvt[ci][:, :],
                            in_=wv_d.ap()[ci * 128 : (ci + 1) * 128, :],
                        )
                    for tt in range(NT):
                        pv = pgv_pool.tile([128, 512], F32, tag="pv", name=f"pv{tt}")
                        for ci in range(1 if "qkvlite" in opts else NCT):
                            nc.tensor.matmul(
                                pv[:, :],
                                xt[ci][:, tt * 128 : (tt + 1) * 128],
                                wvt[ci][:, :],
                                start=(ci == 0),
                                stop=(ci == NCT - 1) or "qkvlite" in opts,
                            )
                        v3 = vt[tt].rearrange("p (l c) -> p l c", c=65)
                        nc.vector.tensor_copy(
                            v3[:, :, 0:64],
                            pv[:, :].rearrange("p (l c) -> p l c", c=64),
                        )
                        nc.vector.memset(v3[:, :, 64:65], 1.0)

            # ---------------- Phase B: attention per head ----------------
            with tc.tile_pool(name="wp", bufs=1) as wp_pool:
                wpt = [wp_pool.tile([128, C], F32R, tag=f"wp{p}", name=f"wp{p}") for p in range(4)]
                for p in range(4):
                    nc.sync.dma_start(
                        out=wpt[p][:, :], in_=wp_d.ap()[p * 128 : (p + 1) * 128, :]
                    )

                with (
                    tc.tile_pool(name="apool", bufs=2) as a_pool,
                    tc.tile_pool(name="sg", bufs=1, space="PSUM") as sg_pool,
                    tc.tile_pool(name="yq", bufs=2, space="PSUM") as yq_pool,
                    tc.tile_pool(name="rbp", bufs=2, space="PSUM") as rbp_pool,
                ):
                    for h in range(0 if "onlyA" in opts else NHL):
                        jq = h // 2
                        jk = 4 + h // 2
                        off = (h % 2) * 64
                        # -- pass 1: S^T tiles, exp, causal mask --
                        a_tiles = []
                        for k in range(NK):
                            width = T - 128 * k
                            sg = sg_pool.tile([128, T], F32, tag="sg", name=f"sg{h}_{k}")
                            for qc in range(k // 4, (k // 4 + 1) if "stlite" in opts else NQ):
                                if "nost" in opts:
                                    break
                                q0 = max(qc * 512, k * 128)
                                q1 = (qc + 1) * 512
                                nc.tensor.matmul(
                                    sg[:, q0:q1],
                                    qkt[jk][off : off + 64, k * 128 : (k + 1) * 128],
                                    qkt[jq][off : off + 64, q0:q1],
                                    start=True,
                                    stop=True,
                                )
                            at = a_pool.tile([128, width], F16, tag=f"a{k}", name=f"a{h}_{k}")
                            if "noexp" in opts:
                                nc.vector.tensor_copy(at[:, 0:128], sg[:, 128 * k : 128 * k + 128])
                            else:
                                nc.scalar.activation(
                                    at[:, :], sg[:, 128 * k : T], EXP, scale=0.125
                                )
                            if "nomask" not in opts:
                                nc.vector.tensor_mul(
                                    at[:, 0:128], at[:, 0:128], tri_sb[:, :]
                                )
                            a_tiles.append(at)
                        # -- pass 2: AV + denominator + normalize --
                        for qc in range(NQ if "noav" not in opts else 0):
                            yq = yq_pool.tile([65, 512], F32, tag="yq", name=f"yq{h}_{qc}")
                            klast = 0 if "avlite" in opts else (4 * qc + 3)
                            for k in range(klast + 1):
                                vsl = vt[k][:, h * 65 : (h + 1) * 65]
                                if k >= 4 * qc:  # diagonal tile
                                    n = 512 - (128 * k - 512 * qc)
                                    nc.tensor.matmul(
                                        yq[:, 512 - n : 512],
                                        vsl,
                                        a_tiles[k][:, 0:n],
                                        start=(k == 0),
                                        stop=(k == klast),
                                    )
                                else:
                                    c0 = qc * 512 - 128 * k
                                    nc.tensor.matmul(
                                        yq[:, :],
                                        vsl,
                                        a_tiles[k][:, c0 : c0 + 512],
                                        start=(k == 0),
                                        stop=(k == klast),
                                    )
                            if "nonorm" in opts:
                                nc.vector.tensor_copy(
                                    yt[jq][off : off + 64, qc * 512 : (qc + 1) * 512],
                                    yq[0:64, :],
                                )
                            else:
                                r = small_pool.tile([1, 512], F32R, tag="recip", name=f"r{h}_{qc}")
                                with nc.allow_low_precision(reason="f32r is fp32-width"):
                                    nc.vector.reciprocal(r[:, :], yq[64:65, :])
                                rbp = rbp_pool.tile([64, 512], F32, tag="rbp", name=f"rbp{h}_{qc}")
                                nc.tensor.matmul(
                                    rbp[:, :], ones_sb[:, :], r[:, :], start=True, stop=True
                                )
                                rb = small_pool.tile([64, 512], F32, tag="rb", name=f"rb{h}_{qc}")
                                nc.vector.tensor_copy(rb[:, :], rbp[:, :])
                                nc.vector.tensor_mul(
                                    yt[jq][off : off + 64, qc * 512 : (qc + 1) * 512],
                                    yq[0:64, :],
                                    rb[:, :],
                                )

                # ---------------- Phase C: output projection ----------------
                with (
                    tc.tile_pool(name="pj", bufs=4, space="PSUM") as pj_pool,
                    tc.tile_pool(name="ost", bufs=4) as ost_pool,
                ):
                    for tt in range(0 if ("onlyA" in opts or "noC" in opts) else NT):
                        for jc in range(2):
                            pj = pj_pool.tile([128, 512], F32, tag="pj", name=f"pj{tt}_{jc}")
                            for p in range(4):
                                nc.tensor.matmul(
                                    pj[:, :],
                                    yt[p][:, tt * 128 : (tt + 1) * 128],
                                    wpt[p][:, jc * 512 : (jc + 1) * 512],
                                    start=(p == 0),
                                    stop=(p == 3),
                                )
                            ot = ost_pool.tile([128, 512], F32, tag="ost", name=f"ost{tt}_{jc}")
                            nc.scalar.copy(ot[:, :], pj[:, :])
                            nc.sync.dma_start(
                                out=out_d.ap()[
                                    tt * 128 : (tt + 1) * 128,
                                    jc * 512 : (jc + 1) * 512,
                                ],
                                in_=ot[:, :],
                            )

    _split_multi_waits(nc)
    return nc


_CACHED = {}


def _get_program():
    if "nc" not in _CACHED:
        _CACHED["nc"] = _build()
    return _CACHED["nc"]


def _shard_inputs(x, w_qkv, w_proj):
    x = np.ascontiguousarray(x, dtype=np.float32)
    w_qkv = np.ascontiguousarray(w_qkv, dtype=np.float32)
    w_proj = np.ascontiguousarray(w_proj, dtype=np.float32)
    tri = np.triu(np.ones((128, 128), dtype=np.float16))
    ones64 = np.ones((1, 64), dtype=np.float32)
    in_maps = []
    for core in range(8):
        b, g = core // 2, core % 2
        xt = np.ascontiguousarray(x[b].T)
        wqk = np.ascontiguousarray(
            np.concatenate(
                [
                    w_qkv[:, g * 512 : g * 512 + 512],
                    w_qkv[:, 1024 + g * 512 : 1024 + g * 512 + 512],
                ],
                axis=1,
            )
        )
        wv = np.ascontiguousarray(w_qkv[:, 2048 + g * 512 : 2048 + g * 512 + 512])
        wp = np.ascontiguousarray(w_proj[g * 512 : (g + 1) * 512, :])
        in_maps.append(
            {"xt": xt, "wqk": wqk, "wv": wv, "wp": wp, "tri": tri, "ones64": ones64}
        )
    return in_maps


def kernel(x, w_qkv, w_proj, _trace=False, _result_box=None):
    nc = _get_program()
    in_maps = _shard_inputs(x, w_qkv, w_proj)
    res = run_bass_kernel_spmd(nc, in_maps, list(range(8)), trace=_trace)
    if _result_box is not None:
        _result_box.append(res)
    B = x.shape[0]
    out = np.empty((B, T, C), dtype=np.float32)
    for b in range(B):
        out[b] = res.results[2 * b]["out"] + res.results[2 * b + 1]["out"]
    return out
